# revision 21
# baseline (speedup 1.0000x reference)
"""Distributed single-head transformer block on 8 TRN2 NeuronCores (v2).

Sharding: token dim (4096) split 8 ways (512 tokens/core), weights
replicated (host pre-transposes so every matmul contracts over the
partition axis). Attention needs all tokens' K/V: each core computes its
local K^T/V scaled by 16 in fp8, and chunked AllGathers distribute them
while the PE keeps computing.

Key numeric/layout tricks vs v1:
  - All attention-path matmuls run fp8 DoubleRow (2x PE throughput):
    QKV projections, scores, attn@V. Gathered K/V are consumed as fp8
    directly (no bf16 casts). Probs are stored fp8 straight out of the
    Exp activation (scale folds the 16x16 weight scaling).
  - The FFN has NO activation between its two Linears, so it collapses
    into one [D,D] matmul precomputed on the host:
        Mi = W2@W1 + I;  out_pre = h@Mi.T + (W2@b1 + b2)
    with LN0 folded in:  out_pre = rstd0*(Wff@res) + msr0*wcol + cb,
        Wff = Mi*g0[None,:], wcol = -Mi@g0, cb = Mi@b0 + W2@b1 + b2.
    This cuts FFN PE work 8x and FFN weight DMA to 2 MB.
  - Per-token scalars (1/denom, rstd, mu*rstd) are broadcast along
    partitions with a rank-1 PE matmul into PSUM instead of a DRAM
    roundtrip.
  - LN sums use 1/D-valued ones vectors so the stats matmuls produce
    means directly.

Output is outT [D, 512] per core; the host transposes and concatenates.
"""

import numpy as np

P = 128
D = 1024
N = 4096
NCORES = 8
TOK = N // NCORES  # 512 tokens per core
DK = D // P  # 8   feature tiles
MT = TOK // P  # 4   local token tiles
NJ = N // P  # 32  global key tiles
SCALE = 1.0 / float(np.sqrt(D))
WS = 16.0  # fp8 scale for QKV weights (and thus q/k/v activations)
EPS = 1e-5

KSPLIT = [2, 2]  # K AllGather chunks, in token tiles per rank
VSPLIT = [4, 3, 1]  # V AllGather chunks, in feature tiles
MBLK = P * MT * P  # one V feature-tile for one rank, elements

_cache = {}


def _build_nc():
    import concourse.tile as tile
    from concourse import bacc, mybir
    from contextlib import ExitStack

    f32 = mybir.dt.float32
    f32r = mybir.dt.float32r
    bf16 = mybir.dt.bfloat16
    f8 = mybir.dt.float8e4
    Exp = mybir.ActivationFunctionType.Exp
    Sqrt = mybir.ActivationFunctionType.Sqrt
    Ident = mybir.ActivationFunctionType.Identity
    DR = mybir.MatmulPerfMode.DoubleRow

    nc = bacc.Bacc("TRN2", target_bir_lowering=False, debug=False, num_devices=NCORES)

    xT = nc.dram_tensor("xT", [D, TOK], f32, kind="ExternalInput").ap()
    WqT = nc.dram_tensor("WqT", [D, D], f8, kind="ExternalInput").ap()
    WkT = nc.dram_tensor("WkT", [D, D], f8, kind="ExternalInput").ap()
    WvT = nc.dram_tensor("WvT", [D, D], f8, kind="ExternalInput").ap()
    WffT = nc.dram_tensor("WffT", [D, D], bf16, kind="ExternalInput").ap()
    bv16 = nc.dram_tensor("bv16", [D], f32, kind="ExternalInput").ap()
    wcol = nc.dram_tensor("wcol", [D], f32, kind="ExternalInput").ap()
    cbv = nc.dram_tensor("cbv", [D], f32, kind="ExternalInput").ap()
    g1 = nc.dram_tensor("g1", [D], f32, kind="ExternalInput").ap()
    b1n = nc.dram_tensor("b1n", [D], f32, kind="ExternalInput").ap()
    outT = nc.dram_tensor("outT", [D, TOK], f32, kind="ExternalOutput").ap()

    with tile.TileContext(nc) as tc, ExitStack() as ctx:
        dram = ctx.enter_context(tc.tile_pool(name="dram", bufs=1, space="DRAM"))
        consts = ctx.enter_context(tc.tile_pool(name="consts", bufs=1))
        xq = ctx.enter_context(tc.tile_pool(name="xq", bufs=1))
        big = ctx.enter_context(tc.tile_pool(name="big", bufs=1))
        wst = ctx.enter_context(tc.tile_pool(name="wst", bufs=3))
        wv_st = ctx.enter_context(tc.tile_pool(name="wv_st", bufs=2))
        kvst = ctx.enter_context(tc.tile_pool(name="kvst", bufs=3))
        met = ctx.enter_context(tc.tile_pool(name="met", bufs=1))
        vtst = ctx.enter_context(tc.tile_pool(name="vtst", bufs=3))
        ev = ctx.enter_context(tc.tile_pool(name="ev", bufs=4))
        fts = ctx.enter_context(tc.tile_pool(name="fts", bufs=1))
        bcs = ctx.enter_context(tc.tile_pool(name="bcs", bufs=2))
        lns = ctx.enter_context(tc.tile_pool(name="lns", bufs=4))
        ps = ctx.enter_context(tc.tile_pool(name="ps", bufs=4, space="PSUM"))
        bc = ctx.enter_context(tc.tile_pool(name="bc", bufs=2, space="PSUM"))
        pss = ctx.enter_context(tc.tile_pool(name="pss", bufs=2, space="PSUM"))

        kv_in_k, kv_out_k = [], []
        for c, ctiles in enumerate(KSPLIT):
            csz = ctiles * P
            kv_in_k.append(
                dram.tile([D * csz], f8, name=f"kvik{c}", tag=f"kvik{c}")
            )
            kv_out_k.append(
                dram.tile(
                    [NCORES * D * csz],
                    f8,
                    addr_space="Shared",
                    name=f"kvok{c}",
                    tag=f"kvok{c}",
                )
            )
        kv_in_v, kv_out_v = [], []
        for c, cm in enumerate(VSPLIT):
            kv_in_v.append(
                dram.tile([cm * MBLK], f8, name=f"kviv{c}", tag=f"kviv{c}")
            )
            kv_out_v.append(
                dram.tile(
                    [NCORES * cm * MBLK],
                    f8,
                    addr_space="Shared",
                    name=f"kvov{c}",
                    tag=f"kvov{c}",
                )
            )

        # ---- constants -------------------------------------------------
        stage = consts.tile([P, 1], f32)
        nc.vector.memset(stage, 1.0)
        ones_f8 = consts.tile([P, 1], f8)
        nc.vector.tensor_copy(ones_f8, stage)

        # A tiny first AllGather absorbs the collective engine's one-time
        # setup latency (~10us) so the K gather's data starts moving right
        # when the cross-core barrier resolves.
        dcc_stf = consts.tile([1, 64], f32)
        nc.vector.memset(dcc_stf, 1.0)
        dcc_st = consts.tile([1, 64], f8)
        nc.vector.tensor_copy(dcc_st, dcc_stf)
        dcc_in = dram.tile([64], f8, name="dcc_in", tag="dcc_in")
        dcc_out = dram.tile(
            [NCORES * 64], f8, addr_space="Shared", name="dcc_out", tag="dcc_out"
        )
        nc.sync.dma_start(out=dcc_in[:].rearrange("(p f) -> p f", p=1), in_=dcc_st)
        nc.gpsimd.collective_compute(
            "AllGather",
            mybir.AluOpType.bypass,
            replica_groups=[list(range(NCORES))],
            ins=[dcc_in[:]],
            outs=[dcc_out[:]],
        )
        stage2 = consts.tile([P, 1], f32)
        nc.vector.memset(stage2, 1.0 / D)
        onesd_r = consts.tile([P, 1], f32r)
        nc.vector.tensor_copy(onesd_r, stage2)
        onesd_b = consts.tile([P, 1], bf16)
        nc.vector.tensor_copy(onesd_b, stage2)
        ones16_r = consts.tile([1, P], f32)
        nc.vector.memset(ones16_r, 1.0 / WS)
        ones1_r = consts.tile([1, P], f32)
        nc.vector.memset(ones1_r, 1.0)
        eps_sb = consts.tile([1, 1], f32)
        nc.vector.memset(eps_sb, EPS)
        bv_b = consts.tile([P, D], f32)
        nc.gpsimd.dma_start(out=bv_b, in_=bv16[None, :].to_broadcast([P, D]))
        g1_sb = consts.tile([P, DK], f32)
        nc.sync.dma_start(out=g1_sb, in_=g1.rearrange("(m p) -> p m", p=P))
        b1n_sb = consts.tile([P, DK], f32)
        nc.sync.dma_start(out=b1n_sb, in_=b1n.rearrange("(m p) -> p m", p=P))
        wcol_sb = consts.tile([P, DK], f32)
        nc.sync.dma_start(out=wcol_sb, in_=wcol.rearrange("(m p) -> p m", p=P))
        cb_sb = consts.tile([P, DK], f32)
        nc.sync.dma_start(out=cb_sb, in_=cbv.rearrange("(m p) -> p m", p=P))

        # ---- load xT, cast to fp8 --------------------------------------
        # x-tile DMAs fan out across four trigger queues so the load is
        # not serialized on one ring; a throwaway matmul after each cast
        # keeps the PE's activity monitor warm (paced by the DMA arrivals)
        # so the projections start at full clock.
        xT_sb = xq.tile([P, DK, TOK], f32)
        x_f8 = xq.tile([P, DK, TOK], f8)
        xT_re = xT.rearrange("(k p) f -> p k f", p=P)
        dmaq = [nc.sync, nc.scalar]
        for k in range(DK):
            dmaq[k % 2].dma_start(out=xT_sb[:, k, :], in_=xT_re[:, k, :])
        for k in range(DK):
            nc.vector.tensor_copy(x_f8[:, k, :], xT_sb[:, k, :])
            hb = ps.tile([P, TOK], f32, tag="pb")
            nc.tensor.matmul(hb, x_f8[:, k, 0:P], x_f8[:, k, :])

        # ---- K projection (fp8 DoubleRow), then its AllGather ---------
        kT_f8 = xq.tile([P, DK, TOK], f8)
        qT_f8 = xq.tile([P, DK, TOK], f8)

        def _proj(wap, dst):
            wre = wap.rearrange("(k p) m -> p k m", p=P)
            for m in range(DK):
                wt = wst.tile([P, DK, P], f8, tag="w", name=f"wt_{m}")
                nc.sync.dma_start(out=wt, in_=wre[:, :, m * P : (m + 1) * P])
                pt = ps.tile([P, TOK], f32, tag="pb", name=f"pt_{m}")
                for k2 in range(DK // 2):
                    nc.tensor.matmul(
                        pt,
                        wt[:, 2 * k2 : 2 * k2 + 2, :],
                        x_f8[:, 2 * k2 : 2 * k2 + 2, :],
                        start=(k2 == 0),
                        stop=(k2 == DK // 2 - 1),
                        perf_mode=DR,
                    )
                nc.vector.tensor_copy(dst[:, m, :], pt)

        _proj(WkT, kT_f8)
        kt0 = 0
        for c, ctiles in enumerate(KSPLIT):
            csz = ctiles * P
            nc.sync.dma_start(
                out=kv_in_k[c][:].rearrange("(k p f) -> p k f", p=P, k=DK),
                in_=kT_f8[:, :, kt0 * P : kt0 * P + csz],
            )
            nc.gpsimd.collective_compute(
                "AllGather",
                mybir.AluOpType.bypass,
                replica_groups=[list(range(NCORES))],
                ins=[kv_in_k[c][:]],
                outs=[kv_out_k[c][:]],
            )
            kt0 += ctiles

        # ---- V projection (fp8 DoubleRow), then its AllGather ---------
        v_sb = xq.tile([P, MT, D], f8)
        wvre = WvT.rearrange("(k p) m -> p k m", p=P)
        for n2 in range(2):
            wvt = wv_st.tile([P, DK, TOK], f8, tag="wv")
            nc.sync.dma_start(out=wvt, in_=wvre[:, :, n2 * TOK : (n2 + 1) * TOK])
            for t in range(MT):
                pt = ps.tile([P, TOK], f32, tag="pb")
                for k2 in range(DK // 2):
                    nc.tensor.matmul(
                        pt,
                        x_f8[:, 2 * k2 : 2 * k2 + 2, t * P : (t + 1) * P],
                        wvt[:, 2 * k2 : 2 * k2 + 2, :],
                        start=(k2 == 0),
                        stop=(k2 == DK // 2 - 1),
                        perf_mode=DR,
                    )
                nc.vector.tensor_add(
                    v_sb[:, t, n2 * TOK : (n2 + 1) * TOK],
                    pt,
                    bv_b[:, n2 * TOK : (n2 + 1) * TOK],
                )
        m0 = 0
        for c, cm in enumerate(VSPLIT):
            for mi in range(cm):
                m = m0 + mi
                nc.sync.dma_start(
                    out=kv_in_v[c][mi * MBLK : (mi + 1) * MBLK].rearrange(
                        "(p t f) -> p t f", p=P, t=MT
                    ),
                    in_=v_sb[:, :, m * P : (m + 1) * P],
                )
            nc.gpsimd.collective_compute(
                "AllGather",
                mybir.AluOpType.bypass,
                replica_groups=[list(range(NCORES))],
                ins=[kv_in_v[c][:]],
                outs=[kv_out_v[c][:]],
            )
            m0 += cm

        _proj(WqT, qT_f8)

        # prefetch the collapsed-FFN weights during the attention phase
        wff_sb = fts.tile([P, DK * DK, P], bf16)
        wffre = WffT.rearrange("(k p) m -> p k m", p=P)
        for m in range(DK):
            nc.sync.dma_start(
                out=wff_sb[:, m * DK : (m + 1) * DK, :],
                in_=wffre[:, :, m * P : (m + 1) * P],
            )

        # DMA-paced heater: a self-chained SBUF->SBUF copy loop on the
        # (otherwise idle) gpsimd queue releases one throwaway matmul per
        # ~1.5us, so the PE's activity monitor stays at full clock through
        # the AllGather waits instead of re-throttling to half rate.
        sc0 = met.tile([P, DK, TOK], f8, name="sc0", tag="sc0")
        sc1 = met.tile([P, DK, TOK], f8, name="sc1", tag="sc1")
        scs = [sc0, sc1]

        def heat_window(seed_src, seed_tiles, links):
            nc.gpsimd.dma_start(out=scs[0][:, 0:seed_tiles, :], in_=seed_src)
            for i in range(links):
                src, dst = scs[i % 2], scs[(i + 1) % 2]
                nc.gpsimd.dma_start(out=dst, in_=src)
                hb = ps.tile([P, TOK], f32, tag="pb")
                nc.tensor.matmul(hb, dst[:, 0, 0:P], dst[:, 0, :])

        heat_window(qT_f8[:, 0:DK, :], DK, 8)

        # ---- scores S^T -> exp -> fp8 probs, denominator interleaved --
        pT_sb = big.tile([P, NJ, TOK], f8, tag="big")
        psd = pss.tile([1, TOK], f32, tag="psm")
        tbase = 0
        for c, ctiles in enumerate(KSPLIT):
            csz = ctiles * P
            for r in range(NCORES):
                ktb = kvst.tile([P, DK, csz], f8, tag=f"kt{c}", name=f"ktb{c}_{r}")
                nc.sync.dma_start(
                    out=ktb,
                    in_=kv_out_k[c][r * D * csz : (r + 1) * D * csz].rearrange(
                        "(k p f) -> p k f", p=P, k=DK
                    ),
                )
                for mj in range(ctiles):
                    kt_i = r * MT + tbase + mj
                    pt = ps.tile([P, TOK], f32, tag="pb")
                    for k2 in range(DK // 2):
                        nc.tensor.matmul(
                            pt,
                            ktb[:, 2 * k2 : 2 * k2 + 2, mj * P : (mj + 1) * P],
                            qT_f8[:, 2 * k2 : 2 * k2 + 2, :],
                            start=(k2 == 0),
                            stop=(k2 == DK // 2 - 1),
                            perf_mode=DR,
                        )
                    nc.scalar.activation(
                        pT_sb[:, kt_i, :], pt, Exp, bias=0.0, scale=SCALE / (WS * WS)
                    )
                    nc.tensor.matmul(
                        psd,
                        ones_f8,
                        pT_sb[:, kt_i, :],
                        start=(c == 0 and r == 0 and mj == 0),
                        stop=(
                            c == len(KSPLIT) - 1
                            and r == NCORES - 1
                            and mj == ctiles - 1
                        ),
                    )
            tbase += ctiles
            if c == 0:
                # keep the PE warm while the next K chunk's gather lands
                last_kt = (NCORES - 1) * MT + tbase - 1
                heat_window(pT_sb[:, last_kt - 1 : last_kt + 1, :], 2, 5)

        # rden = 1/(16*denom), broadcast along partitions via rank-1 MM
        rden = lns.tile([1, TOK], f32, tag="ln")
        nc.vector.reciprocal(rden, psd)
        rden_bp = bc.tile([P, TOK], f32, tag="bc")
        nc.tensor.matmul(rden_bp, ones16_r, rden)
        rden_sb = bcs.tile([P, TOK], f32, tag="bcs")
        nc.scalar.copy(rden_sb, rden_bp)
        # keep the PE warm until the first V chunk's gather lands
        heat_window(pT_sb[:, NJ - 2 : NJ, :], 2, 6)

        # ---- attention output attnT = V.T @ P^T (fp8 DR), + residual --
        # After each V feature-chunk completes its residual tiles, the
        # corresponding k-slices of the collapsed FFN run immediately
        # (accumulated in SBUF), so only the last chunk's slice remains
        # after the final AllGather lands.
        resb = [
            fts.tile([P, TOK], bf16, name=f"resb{m}", tag=f"resb{m}")
            for m in range(DK)
        ]
        accf = xq.tile([P, DK, TOK], f32r)
        psm0 = pss.tile([1, TOK], f32, tag="psm")
        psq0 = pss.tile([1, TOK], f32, tag="psm")
        m0 = 0
        for c, cm in enumerate(VSPLIT):
            for mi in range(cm):
                m = m0 + mi
                pt = ps.tile([P, TOK], f32, tag="pb")
                for r in range(NCORES):
                    vt = vtst.tile([P, MT, P], f8, tag="vt")
                    nc.sync.dma_start(
                        out=vt,
                        in_=kv_out_v[c][
                            (r * cm + mi) * MBLK : (r * cm + mi + 1) * MBLK
                        ].rearrange("(p t f) -> p t f", p=P, t=MT),
                    )
                    for tp in range(MT // 2):
                        kt_i = r * MT + 2 * tp
                        nc.tensor.matmul(
                            pt,
                            vt[:, 2 * tp : 2 * tp + 2, :],
                            pT_sb[:, kt_i : kt_i + 2, :],
                            start=(r == 0 and tp == 0),
                            stop=(r == NCORES - 1 and tp == MT // 2 - 1),
                            perf_mode=DR,
                        )
                tmp = ev.tile([P, TOK], f32, tag="ev")
                nc.vector.tensor_mul(tmp, pt, rden_sb)
                nc.vector.tensor_add(resb[m][:], tmp, xT_sb[:, m, :])
                sq = ev.tile([P, TOK], bf16, tag="evb")
                nc.vector.tensor_mul(sq, resb[m][:], resb[m][:])
                nc.tensor.matmul(
                    psm0, onesd_b, resb[m][:], start=(m == 0), stop=(m == DK - 1)
                )
                nc.tensor.matmul(
                    psq0, onesd_b, sq, start=(m == 0), stop=(m == DK - 1)
                )
            # FFN k-slices for this chunk's freshly finished resb tiles
            for m in range(DK):
                pt2 = ps.tile([P, TOK], f32, tag="pb")
                for ki in range(cm):
                    k = m0 + ki
                    nc.tensor.matmul(
                        pt2,
                        wff_sb[:, m * DK + k, :],
                        resb[k][:],
                        start=(ki == 0),
                        stop=(ki == cm - 1),
                    )
                if c == 0:
                    nc.vector.tensor_copy(accf[:, m, :], pt2)
                else:
                    nc.vector.tensor_add(accf[:, m, :], accf[:, m, :], pt2)
            m0 += cm

        # ---- LN0 stats finalize: rstd0 / mu0*rstd0 broadcasts ---------
        def ln_chain(psm, psq):
            mu2 = lns.tile([1, TOK], f32, tag="ln")
            nc.scalar.square(mu2, psm)
            var = lns.tile([1, TOK], f32, tag="ln")
            nc.vector.tensor_sub(var, psq, mu2)
            std = lns.tile([1, TOK], f32, tag="ln")
            nc.scalar.activation(std, var, Sqrt, bias=eps_sb[:])
            rstd = lns.tile([1, TOK], f32, tag="ln")
            nc.vector.reciprocal(rstd, std)
            msr = lns.tile([1, TOK], f32, tag="ln")
            nc.vector.tensor_mul(msr, psm, rstd)
            rstd_bp = bc.tile([P, TOK], f32, tag="bc")
            nc.tensor.matmul(rstd_bp, ones1_r, rstd)
            msr_bp = bc.tile([P, TOK], f32, tag="bc")
            nc.tensor.matmul(msr_bp, ones1_r, msr)
            return rstd_bp, msr_bp

        rstd0_bp, msr0_bp = ln_chain(psm0, psq0)
        rstd0_sb = bcs.tile([P, TOK], f32, tag="bcs")
        nc.scalar.copy(rstd0_sb, rstd0_bp)

        # ---- FFN finalize (accf already holds Wff@res) + LN1 stats ----
        psm1 = pss.tile([1, TOK], f32, tag="psm")
        psq1 = pss.tile([1, TOK], f32, tag="psm")
        for m in range(DK):
            u = ev.tile([P, TOK], f32, tag="ev")
            nc.scalar.activation(
                u,
                msr0_bp,
                Ident,
                bias=cb_sb[:, m : m + 1],
                scale=wcol_sb[:, m : m + 1],
            )
            nc.vector.tensor_mul(accf[:, m, :], accf[:, m, :], rstd0_sb)
            nc.vector.tensor_add(accf[:, m, :], accf[:, m, :], u)
            sq = ev.tile([P, TOK], bf16, tag="evb")
            nc.vector.tensor_mul(sq, accf[:, m, :], accf[:, m, :])
            nc.tensor.matmul(
                psm1, onesd_r, accf[:, m, :], start=(m == 0), stop=(m == DK - 1)
            )
            nc.tensor.matmul(
                psq1, onesd_b, sq, start=(m == 0), stop=(m == DK - 1)
            )

        # ---- final layernorm + writeback ------------------------------
        rstd1_bp, msr1_bp = ln_chain(psm1, psq1)
        out_re = outT.rearrange("(m p) f -> p m f", p=P)
        for m in range(DK):
            t1 = ev.tile([P, TOK], f32, tag="ev")
            nc.vector.tensor_mul(t1, accf[:, m, :], rstd1_bp)
            t2 = ev.tile([P, TOK], f32, tag="ev")
            nc.vector.tensor_sub(t2, t1, msr1_bp)
            ot = ev.tile([P, TOK], f32, tag="ev")
            nc.scalar.activation(
                ot,
                t2,
                Ident,
                bias=b1n_sb[:, m : m + 1],
                scale=g1_sb[:, m : m + 1],
            )
            dmaq[m % 2].dma_start(out=out_re[:, m, :], in_=ot)

    nc.finalize()
    return nc


def _get_nc():
    if "nc" not in _cache:
        _cache["nc"] = _build_nc()
    return _cache["nc"]


def _make_in_maps(inputs):
    import ml_dtypes

    bf = ml_dtypes.bfloat16
    f8 = ml_dtypes.float8_e4m3
    x = np.ascontiguousarray(np.asarray(inputs["x"], dtype=np.float32))
    Wq = np.asarray(inputs["Wq"], np.float64)
    Wk = np.asarray(inputs["Wk"], np.float64)
    Wv = np.asarray(inputs["Wv"], np.float64)
    W1 = np.asarray(inputs["W1"], np.float64)
    W2 = np.asarray(inputs["W2"], np.float64)
    g0 = np.asarray(inputs["g0"], np.float64)
    b0 = np.asarray(inputs["b0"], np.float64)
    b1 = np.asarray(inputs["b1"], np.float64)
    b2 = np.asarray(inputs["b2"], np.float64)
    bv = np.asarray(inputs["bv"], np.float64)
    # FFN has no activation between the Linears: collapse + LN0 fold.
    Mi = W2 @ W1 + np.eye(D)
    Wff = Mi * g0[None, :]

    def to_f8(a):
        return np.ascontiguousarray(
            np.clip(a, -240.0, 240.0).astype(np.float32)
        ).astype(f8)

    shared = {
        "WqT": to_f8((WS * Wq).T),
        "WkT": to_f8((WS * Wk).T),
        "WvT": to_f8((WS * Wv).T),
        "WffT": np.ascontiguousarray(Wff.T.astype(np.float32)).astype(bf),
        "bv16": np.ascontiguousarray((WS * bv).astype(np.float32)),
        "wcol": np.ascontiguousarray((-(Mi @ g0)).astype(np.float32)),
        "cbv": np.ascontiguousarray((Mi @ b0 + W2 @ b1 + b2).astype(np.float32)),
        "g1": np.ascontiguousarray(np.asarray(inputs["g1"], np.float32)),
        "b1n": np.ascontiguousarray(np.asarray(inputs["b1n"], np.float32)),
    }
    in_maps = []
    for c in range(NCORES):
        m = dict(shared)
        m["xT"] = np.ascontiguousarray(x[c * TOK : (c + 1) * TOK, :].T)
        in_maps.append(m)
    return in_maps


def _assemble(res):
    out = np.empty((N, D), dtype=np.float32)
    for c in range(NCORES):
        out[c * TOK : (c + 1) * TOK, :] = res.results[c]["outT"].T
    return out


def kernel(**inputs):
    from concourse import bass_utils

    nc = _get_nc()
    res = bass_utils.run_bass_kernel_spmd(
        nc, _make_in_maps(inputs), core_ids=list(range(NCORES)), trace=False
    )
    return _assemble(res)


def run_traced(inputs):
    """Like kernel() but with NTFF tracing; returns (out, exec_time_ns, results)."""
    import hookshim

    hookshim.install()
    from concourse import bass_utils

    nc = _get_nc()
    res = bass_utils.run_bass_kernel_spmd(
        nc, _make_in_maps(inputs), core_ids=list(range(NCORES)), trace=True
    )
    return _assemble(res), res.exec_time_ns, res


# revision 27
# speedup vs baseline: 1.2389x; 1.2389x over previous
"""Distributed single-head transformer block on 8 TRN2 NeuronCores (v2).

Sharding: token dim (4096) split 8 ways (512 tokens/core), weights
replicated (host pre-transposes so every matmul contracts over the
partition axis). Attention needs all tokens' K/V: each core computes its
local K^T/V scaled by 16 in fp8, and chunked AllGathers distribute them
while the PE keeps computing.

Key numeric/layout tricks vs v1:
  - All attention-path matmuls run fp8 DoubleRow (2x PE throughput):
    QKV projections, scores, attn@V. Gathered K/V are consumed as fp8
    directly (no bf16 casts). Probs are stored fp8 straight out of the
    Exp activation (scale folds the 16x16 weight scaling).
  - The FFN has NO activation between its two Linears, so it collapses
    into one [D,D] matmul precomputed on the host:
        Mi = W2@W1 + I;  out_pre = h@Mi.T + (W2@b1 + b2)
    with LN0 folded in:  out_pre = rstd0*(Wff@res) + msr0*wcol + cb,
        Wff = Mi*g0[None,:], wcol = -Mi@g0, cb = Mi@b0 + W2@b1 + b2.
    This cuts FFN PE work 8x and FFN weight DMA to 2 MB.
  - Per-token scalars (1/denom, rstd, mu*rstd) are broadcast along
    partitions with a rank-1 PE matmul into PSUM instead of a DRAM
    roundtrip.
  - LN sums use 1/D-valued ones vectors so the stats matmuls produce
    means directly.

Output is outT [D, 512] per core; the host transposes and concatenates.
"""

import numpy as np

P = 128
D = 1024
N = 4096
NCORES = 8
TOK = N // NCORES  # 512 tokens per core
DK = D // P  # 8   feature tiles
MT = TOK // P  # 4   local token tiles
NJ = N // P  # 32  global key tiles
SCALE = 1.0 / float(np.sqrt(D))
WS = 16.0  # fp8 scale for QKV weights (and thus q/k/v activations)
EPS = 1e-5

KSPLIT = [2, 2]  # K AllGather chunks, in token tiles per rank
VSPLIT = [4, 3, 1]  # V AllGather chunks, in feature tiles
MBLK = P * MT * P  # one V feature-tile for one rank, elements

_cache = {}


def _build_nc():
    import concourse.tile as tile
    from concourse import bacc, mybir
    from contextlib import ExitStack

    f32 = mybir.dt.float32
    f32r = mybir.dt.float32r
    bf16 = mybir.dt.bfloat16
    f8 = mybir.dt.float8e4
    Exp = mybir.ActivationFunctionType.Exp
    Sqrt = mybir.ActivationFunctionType.Sqrt
    Ident = mybir.ActivationFunctionType.Identity
    DR = mybir.MatmulPerfMode.DoubleRow

    nc = bacc.Bacc("TRN2", target_bir_lowering=False, debug=False, num_devices=NCORES)

    xT = nc.dram_tensor("xT", [D, TOK], f32, kind="ExternalInput").ap()
    WqT = nc.dram_tensor("WqT", [D, D], f8, kind="ExternalInput").ap()
    WkT = nc.dram_tensor("WkT", [D, D], f8, kind="ExternalInput").ap()
    WvT = nc.dram_tensor("WvT", [D, D], f8, kind="ExternalInput").ap()
    WffT = nc.dram_tensor("WffT", [D, D], bf16, kind="ExternalInput").ap()
    bv16 = nc.dram_tensor("bv16", [D], f32, kind="ExternalInput").ap()
    wcol = nc.dram_tensor("wcol", [D], f32, kind="ExternalInput").ap()
    cbv = nc.dram_tensor("cbv", [D], f32, kind="ExternalInput").ap()
    g1 = nc.dram_tensor("g1", [D], f32, kind="ExternalInput").ap()
    b1n = nc.dram_tensor("b1n", [D], f32, kind="ExternalInput").ap()
    outT = nc.dram_tensor("outT", [D, TOK], f32, kind="ExternalOutput").ap()

    with tile.TileContext(nc) as tc, ExitStack() as ctx:
        dram = ctx.enter_context(tc.tile_pool(name="dram", bufs=1, space="DRAM"))
        consts = ctx.enter_context(tc.tile_pool(name="consts", bufs=1))
        xq = ctx.enter_context(tc.tile_pool(name="xq", bufs=1))
        big = ctx.enter_context(tc.tile_pool(name="big", bufs=1))
        wst = ctx.enter_context(tc.tile_pool(name="wst", bufs=3))
        wv_st = ctx.enter_context(tc.tile_pool(name="wv_st", bufs=2))
        kvst = ctx.enter_context(tc.tile_pool(name="kvst", bufs=3))
        vtst = ctx.enter_context(tc.tile_pool(name="vtst", bufs=3))
        ev = ctx.enter_context(tc.tile_pool(name="ev", bufs=4))
        fts = ctx.enter_context(tc.tile_pool(name="fts", bufs=1))
        bcs = ctx.enter_context(tc.tile_pool(name="bcs", bufs=2))
        lns = ctx.enter_context(tc.tile_pool(name="lns", bufs=4))
        ps = ctx.enter_context(tc.tile_pool(name="ps", bufs=4, space="PSUM"))
        bc = ctx.enter_context(tc.tile_pool(name="bc", bufs=2, space="PSUM"))
        pss = ctx.enter_context(tc.tile_pool(name="pss", bufs=2, space="PSUM"))

        kv_in_k, kv_out_k = [], []
        for c, ctiles in enumerate(KSPLIT):
            csz = ctiles * P
            kv_in_k.append(
                dram.tile([D * csz], f8, name=f"kvik{c}", tag=f"kvik{c}")
            )
            kv_out_k.append(
                dram.tile(
                    [NCORES * D * csz],
                    f8,
                    addr_space="Shared",
                    name=f"kvok{c}",
                    tag=f"kvok{c}",
                )
            )
        kv_in_v, kv_out_v = [], []
        for c, cm in enumerate(VSPLIT):
            kv_in_v.append(
                dram.tile([cm * MBLK], f8, name=f"kviv{c}", tag=f"kviv{c}")
            )
            kv_out_v.append(
                dram.tile(
                    [NCORES * cm * MBLK],
                    f8,
                    addr_space="Shared",
                    name=f"kvov{c}",
                    tag=f"kvov{c}",
                )
            )

        # ---- constants -------------------------------------------------
        stage = consts.tile([P, 1], f32)
        nc.vector.memset(stage, 1.0)
        ones_f8 = consts.tile([P, 1], f8)
        nc.vector.tensor_copy(ones_f8, stage)


        stage2 = consts.tile([P, 1], f32)
        nc.vector.memset(stage2, 1.0 / D)
        onesd_r = consts.tile([P, 1], f32r)
        nc.vector.tensor_copy(onesd_r, stage2)
        onesd_b = consts.tile([P, 1], bf16)
        nc.vector.tensor_copy(onesd_b, stage2)
        ones16_r = consts.tile([1, P], f32)
        nc.vector.memset(ones16_r, 1.0 / WS)
        ones1_r = consts.tile([1, P], f32)
        nc.vector.memset(ones1_r, 1.0)
        eps_sb = consts.tile([1, 1], f32)
        nc.vector.memset(eps_sb, EPS)
        bv_b = consts.tile([P, D], f32)
        nc.gpsimd.dma_start(out=bv_b, in_=bv16[None, :].to_broadcast([P, D]))
        g1_sb = consts.tile([P, DK], f32)
        nc.sync.dma_start(out=g1_sb, in_=g1.rearrange("(m p) -> p m", p=P))
        b1n_sb = consts.tile([P, DK], f32)
        nc.sync.dma_start(out=b1n_sb, in_=b1n.rearrange("(m p) -> p m", p=P))
        wcol_sb = consts.tile([P, DK], f32)
        nc.sync.dma_start(out=wcol_sb, in_=wcol.rearrange("(m p) -> p m", p=P))
        cb_sb = consts.tile([P, DK], f32)
        nc.sync.dma_start(out=cb_sb, in_=cbv.rearrange("(m p) -> p m", p=P))

        # ---- load xT, cast to fp8 --------------------------------------
        # x-tile DMAs fan out across four trigger queues so the load is
        # not serialized on one ring; a throwaway matmul after each cast
        # keeps the PE's activity monitor warm (paced by the DMA arrivals)
        # so the projections start at full clock.
        xT_sb = xq.tile([P, DK, TOK], f32)
        x_f8 = xq.tile([P, DK, TOK], f8)
        xT_re = xT.rearrange("(k p) f -> p k f", p=P)
        dmaq = [nc.sync, nc.scalar]
        for k in range(DK):
            dmaq[k % 2].dma_start(out=xT_sb[:, k, :], in_=xT_re[:, k, :])
        for k in range(DK):
            nc.vector.tensor_copy(x_f8[:, k, :], xT_sb[:, k, :])
            hb = ps.tile([P, TOK], f32, tag="pb")
            nc.tensor.matmul(hb, x_f8[:, k, 0:P], x_f8[:, k, :])

        # ---- K projection (fp8 DoubleRow), then its AllGather ---------
        kT_f8 = xq.tile([P, DK, TOK], f8)
        qT_f8 = xq.tile([P, DK, TOK], f8)

        def _proj(wap, dst):
            wre = wap.rearrange("(k p) m -> p k m", p=P)
            for m in range(DK):
                wt = wst.tile([P, DK, P], f8, tag="w", name=f"wt_{m}")
                nc.sync.dma_start(out=wt, in_=wre[:, :, m * P : (m + 1) * P])
                pt = ps.tile([P, TOK], f32, tag="pb", name=f"pt_{m}")
                for k2 in range(DK // 2):
                    nc.tensor.matmul(
                        pt,
                        wt[:, 2 * k2 : 2 * k2 + 2, :],
                        x_f8[:, 2 * k2 : 2 * k2 + 2, :],
                        start=(k2 == 0),
                        stop=(k2 == DK // 2 - 1),
                        perf_mode=DR,
                    )
                nc.vector.tensor_copy(dst[:, m, :], pt)

        _proj(WkT, kT_f8)
        kt0 = 0
        for c, ctiles in enumerate(KSPLIT):
            csz = ctiles * P
            nc.sync.dma_start(
                out=kv_in_k[c][:].rearrange("(k p f) -> p k f", p=P, k=DK),
                in_=kT_f8[:, :, kt0 * P : kt0 * P + csz],
            )
            nc.gpsimd.collective_compute(
                "AllGather",
                mybir.AluOpType.bypass,
                replica_groups=[list(range(NCORES))],
                ins=[kv_in_k[c][:]],
                outs=[kv_out_k[c][:]],
            )
            kt0 += ctiles

        # ---- V projection (fp8 DoubleRow), then its AllGather ---------
        v_sb = xq.tile([P, MT, D], f8)
        wvre = WvT.rearrange("(k p) m -> p k m", p=P)
        for n2 in range(2):
            wvt = wv_st.tile([P, DK, TOK], f8, tag="wv")
            nc.sync.dma_start(out=wvt, in_=wvre[:, :, n2 * TOK : (n2 + 1) * TOK])
            for t in range(MT):
                pt = ps.tile([P, TOK], f32, tag="pb")
                for k2 in range(DK // 2):
                    nc.tensor.matmul(
                        pt,
                        x_f8[:, 2 * k2 : 2 * k2 + 2, t * P : (t + 1) * P],
                        wvt[:, 2 * k2 : 2 * k2 + 2, :],
                        start=(k2 == 0),
                        stop=(k2 == DK // 2 - 1),
                        perf_mode=DR,
                    )
                nc.vector.tensor_add(
                    v_sb[:, t, n2 * TOK : (n2 + 1) * TOK],
                    pt,
                    bv_b[:, n2 * TOK : (n2 + 1) * TOK],
                )
        m0 = 0
        for c, cm in enumerate(VSPLIT):
            for mi in range(cm):
                m = m0 + mi
                nc.sync.dma_start(
                    out=kv_in_v[c][mi * MBLK : (mi + 1) * MBLK].rearrange(
                        "(p t f) -> p t f", p=P, t=MT
                    ),
                    in_=v_sb[:, :, m * P : (m + 1) * P],
                )
            nc.gpsimd.collective_compute(
                "AllGather",
                mybir.AluOpType.bypass,
                replica_groups=[list(range(NCORES))],
                ins=[kv_in_v[c][:]],
                outs=[kv_out_v[c][:]],
            )
            m0 += cm

        _proj(WqT, qT_f8)

        # prefetch the collapsed-FFN weights during the attention phase
        wff_sb = fts.tile([P, DK * DK, P], bf16)
        wffre = WffT.rearrange("(k p) m -> p k m", p=P)
        for m in range(DK):
            nc.sync.dma_start(
                out=wff_sb[:, m * DK : (m + 1) * DK, :],
                in_=wffre[:, :, m * P : (m + 1) * P],
            )



        # ---- scores S^T -> exp -> fp8 probs, denominator interleaved --
        pT_sb = big.tile([P, NJ, TOK], f8, tag="big")
        psd = pss.tile([1, TOK], f32, tag="psm")
        tbase = 0
        for c, ctiles in enumerate(KSPLIT):
            csz = ctiles * P
            for r in range(NCORES):
                ktb = kvst.tile([P, DK, csz], f8, tag=f"kt{c}", name=f"ktb{c}_{r}")
                nc.sync.dma_start(
                    out=ktb,
                    in_=kv_out_k[c][r * D * csz : (r + 1) * D * csz].rearrange(
                        "(k p f) -> p k f", p=P, k=DK
                    ),
                )
                for mj in range(ctiles):
                    kt_i = r * MT + tbase + mj
                    pt = ps.tile([P, TOK], f32, tag="pb")
                    for k2 in range(DK // 2):
                        nc.tensor.matmul(
                            pt,
                            ktb[:, 2 * k2 : 2 * k2 + 2, mj * P : (mj + 1) * P],
                            qT_f8[:, 2 * k2 : 2 * k2 + 2, :],
                            start=(k2 == 0),
                            stop=(k2 == DK // 2 - 1),
                            perf_mode=DR,
                        )
                    nc.scalar.activation(
                        pT_sb[:, kt_i, :], pt, Exp, bias=0.0, scale=SCALE / (WS * WS)
                    )
                    nc.tensor.matmul(
                        psd,
                        ones_f8,
                        pT_sb[:, kt_i, :],
                        start=(c == 0 and r == 0 and mj == 0),
                        stop=(
                            c == len(KSPLIT) - 1
                            and r == NCORES - 1
                            and mj == ctiles - 1
                        ),
                    )
            tbase += ctiles

        # rden = 1/(16*denom), broadcast along partitions via rank-1 MM
        rden = lns.tile([1, TOK], f32, tag="ln")
        nc.vector.reciprocal(rden, psd)
        rden_bp = bc.tile([P, TOK], f32, tag="bc")
        nc.tensor.matmul(rden_bp, ones16_r, rden)
        rden_sb = bcs.tile([P, TOK], f32, tag="bcs")
        nc.scalar.copy(rden_sb, rden_bp)

        # ---- attention output attnT = V.T @ P^T (fp8 DR), + residual --
        # After each V feature-chunk completes its residual tiles, the
        # corresponding k-slices of the collapsed FFN run immediately
        # (accumulated in SBUF), so only the last chunk's slice remains
        # after the final AllGather lands.
        resb = [
            fts.tile([P, TOK], bf16, name=f"resb{m}", tag=f"resb{m}")
            for m in range(DK)
        ]
        accf = xq.tile([P, DK, TOK], f32r)
        psm0 = pss.tile([1, TOK], f32, tag="psm")
        psq0 = pss.tile([1, TOK], f32, tag="psm")
        m0 = 0
        for c, cm in enumerate(VSPLIT):
            for mi in range(cm):
                m = m0 + mi
                pt = ps.tile([P, TOK], f32, tag="pb")
                for r in range(NCORES):
                    vt = vtst.tile([P, MT, P], f8, tag="vt")
                    nc.sync.dma_start(
                        out=vt,
                        in_=kv_out_v[c][
                            (r * cm + mi) * MBLK : (r * cm + mi + 1) * MBLK
                        ].rearrange("(p t f) -> p t f", p=P, t=MT),
                    )
                    for tp in range(MT // 2):
                        kt_i = r * MT + 2 * tp
                        nc.tensor.matmul(
                            pt,
                            vt[:, 2 * tp : 2 * tp + 2, :],
                            pT_sb[:, kt_i : kt_i + 2, :],
                            start=(r == 0 and tp == 0),
                            stop=(r == NCORES - 1 and tp == MT // 2 - 1),
                            perf_mode=DR,
                        )
                tmp = ev.tile([P, TOK], f32, tag="ev")
                nc.vector.tensor_mul(tmp, pt, rden_sb)
                nc.vector.tensor_add(resb[m][:], tmp, xT_sb[:, m, :])
                sq = ev.tile([P, TOK], bf16, tag="evb")
                nc.vector.tensor_mul(sq, resb[m][:], resb[m][:])
                nc.tensor.matmul(
                    psm0, onesd_b, resb[m][:], start=(m == 0), stop=(m == DK - 1)
                )
                nc.tensor.matmul(
                    psq0, onesd_b, sq, start=(m == 0), stop=(m == DK - 1)
                )
            # FFN k-slices for this chunk's freshly finished resb tiles
            for m in range(DK):
                pt2 = ps.tile([P, TOK], f32, tag="pb")
                for ki in range(cm):
                    k = m0 + ki
                    nc.tensor.matmul(
                        pt2,
                        wff_sb[:, m * DK + k, :],
                        resb[k][:],
                        start=(ki == 0),
                        stop=(ki == cm - 1),
                    )
                if c == 0:
                    nc.vector.tensor_copy(accf[:, m, :], pt2)
                else:
                    nc.vector.tensor_add(accf[:, m, :], accf[:, m, :], pt2)
            m0 += cm

        # ---- LN0 stats finalize: rstd0 / mu0*rstd0 broadcasts ---------
        def ln_chain(psm, psq):
            mu2 = lns.tile([1, TOK], f32, tag="ln")
            nc.scalar.square(mu2, psm)
            var = lns.tile([1, TOK], f32, tag="ln")
            nc.vector.tensor_sub(var, psq, mu2)
            std = lns.tile([1, TOK], f32, tag="ln")
            nc.scalar.activation(std, var, Sqrt, bias=eps_sb[:])
            rstd = lns.tile([1, TOK], f32, tag="ln")
            nc.vector.reciprocal(rstd, std)
            msr = lns.tile([1, TOK], f32, tag="ln")
            nc.vector.tensor_mul(msr, psm, rstd)
            rstd_bp = bc.tile([P, TOK], f32, tag="bc")
            nc.tensor.matmul(rstd_bp, ones1_r, rstd)
            msr_bp = bc.tile([P, TOK], f32, tag="bc")
            nc.tensor.matmul(msr_bp, ones1_r, msr)
            return rstd_bp, msr_bp

        rstd0_bp, msr0_bp = ln_chain(psm0, psq0)
        rstd0_sb = bcs.tile([P, TOK], f32, tag="bcs")
        nc.scalar.copy(rstd0_sb, rstd0_bp)

        # ---- FFN finalize (accf already holds Wff@res) + LN1 stats ----
        # out1 = accf*rstd0 + (msr0*wcol + cb), fused into two DVE ops
        psm1 = pss.tile([1, TOK], f32, tag="psm")
        psq1 = pss.tile([1, TOK], f32, tag="psm")
        for m in range(DK):
            t1 = ev.tile([P, TOK], f32, tag="ev")
            nc.vector.tensor_mul(t1, accf[:, m, :], rstd0_sb)
            nc.vector.affine_then_add(
                accf[:, m, :],
                msr0_bp,
                t1,
                wcol_sb[:, m : m + 1],
                cb_sb[:, m : m + 1],
            )
            sq = ev.tile([P, TOK], bf16, tag="evb")
            nc.vector.tensor_mul(sq, accf[:, m, :], accf[:, m, :])
            nc.tensor.matmul(
                psm1, onesd_r, accf[:, m, :], start=(m == 0), stop=(m == DK - 1)
            )
            nc.tensor.matmul(
                psq1, onesd_b, sq, start=(m == 0), stop=(m == DK - 1)
            )

        # ---- final layernorm + writeback ------------------------------
        # out = (accf*g1)*rstd1 + (msr1*(-g1) + b1n), two fused DVE ops
        rstd1_bp, msr1_bp = ln_chain(psm1, psq1)
        ng1_sb = consts.tile([P, DK], f32)
        nc.vector.tensor_scalar_mul(ng1_sb, g1_sb, -1.0)
        out_re = outT.rearrange("(m p) f -> p m f", p=P)
        for m in range(DK):
            t1 = ev.tile([P, TOK], f32, tag="ev")
            a1 = ev.tile([P, 1], f32, tag="a1")
            nc.vector.affine_mul_reduce(
                t1, a1, accf[:, m, :], rstd1_bp, g1_sb[:, m : m + 1], 0.0
            )
            ot = ev.tile([P, TOK], f32, tag="ev")
            nc.vector.affine_then_add(
                ot,
                msr1_bp,
                t1,
                ng1_sb[:, m : m + 1],
                b1n_sb[:, m : m + 1],
            )
            dmaq[m % 2].dma_start(out=out_re[:, m, :], in_=ot)

    nc.finalize()
    return nc


def _get_nc():
    if "nc" not in _cache:
        _cache["nc"] = _build_nc()
    return _cache["nc"]


def _make_in_maps(inputs):
    import ml_dtypes

    bf = ml_dtypes.bfloat16
    f8 = ml_dtypes.float8_e4m3
    x = np.ascontiguousarray(np.asarray(inputs["x"], dtype=np.float32))
    Wq = np.asarray(inputs["Wq"], np.float64)
    Wk = np.asarray(inputs["Wk"], np.float64)
    Wv = np.asarray(inputs["Wv"], np.float64)
    W1 = np.asarray(inputs["W1"], np.float64)
    W2 = np.asarray(inputs["W2"], np.float64)
    g0 = np.asarray(inputs["g0"], np.float64)
    b0 = np.asarray(inputs["b0"], np.float64)
    b1 = np.asarray(inputs["b1"], np.float64)
    b2 = np.asarray(inputs["b2"], np.float64)
    bv = np.asarray(inputs["bv"], np.float64)
    # FFN has no activation between the Linears: collapse + LN0 fold.
    Mi = W2 @ W1 + np.eye(D)
    Wff = Mi * g0[None, :]

    def to_f8(a):
        return np.ascontiguousarray(
            np.clip(a, -240.0, 240.0).astype(np.float32)
        ).astype(f8)

    shared = {
        "WqT": to_f8((WS * Wq).T),
        "WkT": to_f8((WS * Wk).T),
        "WvT": to_f8((WS * Wv).T),
        "WffT": np.ascontiguousarray(Wff.T.astype(np.float32)).astype(bf),
        "bv16": np.ascontiguousarray((WS * bv).astype(np.float32)),
        "wcol": np.ascontiguousarray((-(Mi @ g0)).astype(np.float32)),
        "cbv": np.ascontiguousarray((Mi @ b0 + W2 @ b1 + b2).astype(np.float32)),
        "g1": np.ascontiguousarray(np.asarray(inputs["g1"], np.float32)),
        "b1n": np.ascontiguousarray(np.asarray(inputs["b1n"], np.float32)),
    }
    in_maps = []
    for c in range(NCORES):
        m = dict(shared)
        m["xT"] = np.ascontiguousarray(x[c * TOK : (c + 1) * TOK, :].T)
        in_maps.append(m)
    return in_maps


def _assemble(res):
    out = np.empty((N, D), dtype=np.float32)
    for c in range(NCORES):
        out[c * TOK : (c + 1) * TOK, :] = res.results[c]["outT"].T
    return out


def kernel(**inputs):
    from concourse import bass_utils

    nc = _get_nc()
    res = bass_utils.run_bass_kernel_spmd(
        nc, _make_in_maps(inputs), core_ids=list(range(NCORES)), trace=False
    )
    return _assemble(res)


def run_traced(inputs):
    """Like kernel() but with NTFF tracing; returns (out, exec_time_ns, results)."""
    import hookshim

    hookshim.install()
    from concourse import bass_utils

    nc = _get_nc()
    res = bass_utils.run_bass_kernel_spmd(
        nc, _make_in_maps(inputs), core_ids=list(range(NCORES)), trace=True
    )
    return _assemble(res), res.exec_time_ns, res


# revision 32
# speedup vs baseline: 1.2658x; 1.0217x over previous
"""Distributed single-head transformer block on 8 TRN2 NeuronCores (v2).

Sharding: token dim (4096) split 8 ways (512 tokens/core), weights
replicated (host pre-transposes so every matmul contracts over the
partition axis). Attention needs all tokens' K/V: each core computes its
local K^T/V scaled by 16 in fp8, and chunked AllGathers distribute them
while the PE keeps computing.

Key numeric/layout tricks vs v1:
  - All attention-path matmuls run fp8 DoubleRow (2x PE throughput):
    QKV projections, scores, attn@V. Gathered K/V are consumed as fp8
    directly (no bf16 casts). Probs are stored fp8 straight out of the
    Exp activation (scale folds the 16x16 weight scaling).
  - The FFN has NO activation between its two Linears, so it collapses
    into one [D,D] matmul precomputed on the host:
        Mi = W2@W1 + I;  out_pre = h@Mi.T + (W2@b1 + b2)
    with LN0 folded in:  out_pre = rstd0*(Wff@res) + msr0*wcol + cb,
        Wff = Mi*g0[None,:], wcol = -Mi@g0, cb = Mi@b0 + W2@b1 + b2.
    This cuts FFN PE work 8x and FFN weight DMA to 2 MB.
  - Per-token scalars (1/denom, rstd, mu*rstd) are broadcast along
    partitions with a rank-1 PE matmul into PSUM instead of a DRAM
    roundtrip.
  - LN sums use 1/D-valued ones vectors so the stats matmuls produce
    means directly.

Output is outT [D, 512] per core; the host transposes and concatenates.
"""

import numpy as np

P = 128
D = 1024
N = 4096
NCORES = 8
TOK = N // NCORES  # 512 tokens per core
DK = D // P  # 8   feature tiles
MT = TOK // P  # 4   local token tiles
NJ = N // P  # 32  global key tiles
SCALE = 1.0 / float(np.sqrt(D))
WS = 16.0  # fp8 scale for QKV weights (and thus q/k/v activations)
EPS = 1e-5

KSPLIT = [2, 2]  # K AllGather chunks, in token tiles per rank
VSPLIT = [4, 3, 1]  # V AllGather chunks, in feature tiles
MBLK = P * MT * P  # one V feature-tile for one rank, elements

_cache = {}


def _build_nc():
    import concourse.tile as tile
    from concourse import bacc, mybir
    from contextlib import ExitStack

    f32 = mybir.dt.float32
    f32r = mybir.dt.float32r
    bf16 = mybir.dt.bfloat16
    f8 = mybir.dt.float8e4
    Exp = mybir.ActivationFunctionType.Exp
    Sqrt = mybir.ActivationFunctionType.Sqrt
    Ident = mybir.ActivationFunctionType.Identity
    DR = mybir.MatmulPerfMode.DoubleRow

    nc = bacc.Bacc("TRN2", target_bir_lowering=False, debug=False, num_devices=NCORES)

    xT = nc.dram_tensor("xT", [D, TOK], f32, kind="ExternalInput").ap()
    WqT = nc.dram_tensor("WqT", [D, D], f8, kind="ExternalInput").ap()
    WkT = nc.dram_tensor("WkT", [D, D], f8, kind="ExternalInput").ap()
    WvT = nc.dram_tensor("WvT", [D, D], f8, kind="ExternalInput").ap()
    WffT = nc.dram_tensor("WffT", [D, D], bf16, kind="ExternalInput").ap()
    bv16 = nc.dram_tensor("bv16", [D], f32, kind="ExternalInput").ap()
    wcol = nc.dram_tensor("wcol", [D], f32, kind="ExternalInput").ap()
    cbv = nc.dram_tensor("cbv", [D], f32, kind="ExternalInput").ap()
    g1 = nc.dram_tensor("g1", [D], f32, kind="ExternalInput").ap()
    b1n = nc.dram_tensor("b1n", [D], f32, kind="ExternalInput").ap()
    outT = nc.dram_tensor("outT", [D, TOK], f32, kind="ExternalOutput").ap()

    with tile.TileContext(nc) as tc, ExitStack() as ctx:
        dram = ctx.enter_context(tc.tile_pool(name="dram", bufs=1, space="DRAM"))
        consts = ctx.enter_context(tc.tile_pool(name="consts", bufs=1))
        xq = ctx.enter_context(tc.tile_pool(name="xq", bufs=1))
        big = ctx.enter_context(tc.tile_pool(name="big", bufs=1))
        wst = ctx.enter_context(tc.tile_pool(name="wst", bufs=3))
        wv_st = ctx.enter_context(tc.tile_pool(name="wv_st", bufs=2))
        kvst = ctx.enter_context(tc.tile_pool(name="kvst", bufs=3))
        vtst = ctx.enter_context(tc.tile_pool(name="vtst", bufs=3))
        ev = ctx.enter_context(tc.tile_pool(name="ev", bufs=4))
        fts = ctx.enter_context(tc.tile_pool(name="fts", bufs=1))
        bcs = ctx.enter_context(tc.tile_pool(name="bcs", bufs=2))
        lns = ctx.enter_context(tc.tile_pool(name="lns", bufs=4))
        ps = ctx.enter_context(tc.tile_pool(name="ps", bufs=4, space="PSUM"))
        bc = ctx.enter_context(tc.tile_pool(name="bc", bufs=2, space="PSUM"))
        pss = ctx.enter_context(tc.tile_pool(name="pss", bufs=2, space="PSUM"))

        kv_in_k, kv_out_k = [], []
        for c, ctiles in enumerate(KSPLIT):
            csz = ctiles * P
            kv_in_k.append(
                dram.tile([D * csz], f8, name=f"kvik{c}", tag=f"kvik{c}")
            )
            kv_out_k.append(
                dram.tile(
                    [NCORES * D * csz],
                    f8,
                    addr_space="Shared",
                    name=f"kvok{c}",
                    tag=f"kvok{c}",
                )
            )
        kv_in_v, kv_out_v = [], []
        for c, cm in enumerate(VSPLIT):
            kv_in_v.append(
                dram.tile([cm * MBLK], f8, name=f"kviv{c}", tag=f"kviv{c}")
            )
            kv_out_v.append(
                dram.tile(
                    [NCORES * cm * MBLK],
                    f8,
                    addr_space="Shared",
                    name=f"kvov{c}",
                    tag=f"kvov{c}",
                )
            )

        # ---- constants -------------------------------------------------
        stage = consts.tile([P, 1], f32)
        nc.vector.memset(stage, 1.0)
        ones_f8 = consts.tile([P, 1], f8)
        nc.vector.tensor_copy(ones_f8, stage)


        stage2 = consts.tile([P, 1], f32)
        nc.vector.memset(stage2, 1.0 / D)
        onesd_r = consts.tile([P, 1], f32r)
        nc.vector.tensor_copy(onesd_r, stage2)
        onesd_b = consts.tile([P, 1], bf16)
        nc.vector.tensor_copy(onesd_b, stage2)
        ones16_r = consts.tile([1, P], f32)
        nc.vector.memset(ones16_r, 1.0 / WS)
        ones1_r = consts.tile([1, P], f32)
        nc.vector.memset(ones1_r, 1.0)
        eps_sb = consts.tile([1, 1], f32)
        nc.vector.memset(eps_sb, EPS)
        bv_b = consts.tile([P, D], f32)
        nc.gpsimd.dma_start(out=bv_b, in_=bv16[None, :].to_broadcast([P, D]))
        g1_sb = consts.tile([P, DK], f32)
        nc.sync.dma_start(out=g1_sb, in_=g1.rearrange("(m p) -> p m", p=P))
        b1n_sb = consts.tile([P, DK], f32)
        nc.sync.dma_start(out=b1n_sb, in_=b1n.rearrange("(m p) -> p m", p=P))
        wcol_sb = consts.tile([P, DK], f32)
        nc.sync.dma_start(out=wcol_sb, in_=wcol.rearrange("(m p) -> p m", p=P))
        cb_sb = consts.tile([P, DK], f32)
        nc.sync.dma_start(out=cb_sb, in_=cbv.rearrange("(m p) -> p m", p=P))

        # ---- load xT, cast to fp8 --------------------------------------
        # x-tile DMAs fan out across four trigger queues so the load is
        # not serialized on one ring; a throwaway matmul after each cast
        # keeps the PE's activity monitor warm (paced by the DMA arrivals)
        # so the projections start at full clock.
        xT_sb = xq.tile([P, DK, TOK], f32)
        x_f8 = xq.tile([P, DK, TOK], f8)
        xT_re = xT.rearrange("(k p) f -> p k f", p=P)
        dmaq = [nc.sync, nc.scalar]
        for k in range(DK):
            dmaq[k % 2].dma_start(out=xT_sb[:, k, :], in_=xT_re[:, k, :])
        for k in range(DK):
            nc.vector.tensor_copy(x_f8[:, k, :], xT_sb[:, k, :])
            hb = ps.tile([P, TOK], f32, tag="pb")
            nc.tensor.matmul(hb, x_f8[:, k, 0:P], x_f8[:, k, :])

        # ---- K projection (fp8 DoubleRow), then its AllGather ---------
        kT_f8 = xq.tile([P, DK, TOK], f8)
        qT_f8 = xq.tile([P, DK, TOK], f8)

        def _proj(wap, dst):
            wre = wap.rearrange("(k p) m -> p k m", p=P)
            for m in range(DK):
                wt = wst.tile([P, DK, P], f8, tag="w", name=f"wt_{m}")
                nc.sync.dma_start(out=wt, in_=wre[:, :, m * P : (m + 1) * P])
                pt = ps.tile([P, TOK], f32, tag="pb", name=f"pt_{m}")
                for k2 in range(DK // 2):
                    nc.tensor.matmul(
                        pt,
                        wt[:, 2 * k2 : 2 * k2 + 2, :],
                        x_f8[:, 2 * k2 : 2 * k2 + 2, :],
                        start=(k2 == 0),
                        stop=(k2 == DK // 2 - 1),
                        perf_mode=DR,
                    )
                nc.vector.tensor_copy(dst[:, m, :], pt)

        _proj(WkT, kT_f8)
        kt0 = 0
        for c, ctiles in enumerate(KSPLIT):
            csz = ctiles * P
            nc.sync.dma_start(
                out=kv_in_k[c][:].rearrange("(k p f) -> p k f", p=P, k=DK),
                in_=kT_f8[:, :, kt0 * P : kt0 * P + csz],
            )
            nc.gpsimd.collective_compute(
                "AllGather",
                mybir.AluOpType.bypass,
                replica_groups=[list(range(NCORES))],
                ins=[kv_in_k[c][:]],
                outs=[kv_out_k[c][:]],
            )
            kt0 += ctiles

        # ---- V projection (fp8 DoubleRow), then its AllGather ---------
        v_sb = xq.tile([P, MT, D], f8)
        wvre = WvT.rearrange("(k p) m -> p k m", p=P)
        for n2 in range(2):
            wvt = wv_st.tile([P, DK, TOK], f8, tag="wv")
            nc.sync.dma_start(out=wvt, in_=wvre[:, :, n2 * TOK : (n2 + 1) * TOK])
            for t in range(MT):
                pt = ps.tile([P, TOK], f32, tag="pb")
                for k2 in range(DK // 2):
                    nc.tensor.matmul(
                        pt,
                        x_f8[:, 2 * k2 : 2 * k2 + 2, t * P : (t + 1) * P],
                        wvt[:, 2 * k2 : 2 * k2 + 2, :],
                        start=(k2 == 0),
                        stop=(k2 == DK // 2 - 1),
                        perf_mode=DR,
                    )
                nc.vector.tensor_add(
                    v_sb[:, t, n2 * TOK : (n2 + 1) * TOK],
                    pt,
                    bv_b[:, n2 * TOK : (n2 + 1) * TOK],
                )
        m0 = 0
        for c, cm in enumerate(VSPLIT):
            for mi in range(cm):
                m = m0 + mi
                nc.sync.dma_start(
                    out=kv_in_v[c][mi * MBLK : (mi + 1) * MBLK].rearrange(
                        "(p t f) -> p t f", p=P, t=MT
                    ),
                    in_=v_sb[:, :, m * P : (m + 1) * P],
                )
            nc.gpsimd.collective_compute(
                "AllGather",
                mybir.AluOpType.bypass,
                replica_groups=[list(range(NCORES))],
                ins=[kv_in_v[c][:]],
                outs=[kv_out_v[c][:]],
            )
            m0 += cm

        _proj(WqT, qT_f8)

        # prefetch the collapsed-FFN weights during the attention phase
        wff_sb = fts.tile([P, DK * DK, P], bf16)
        wffre = WffT.rearrange("(k p) m -> p k m", p=P)
        for m in range(DK):
            nc.sync.dma_start(
                out=wff_sb[:, m * DK : (m + 1) * DK, :],
                in_=wffre[:, :, m * P : (m + 1) * P],
            )



        # ---- scores S^T -> exp -> fp8 probs, denominator interleaved --
        pT_sb = big.tile([P, NJ, TOK], f8, tag="big")
        psd = pss.tile([1, TOK], f32, tag="psm")
        tbase = 0
        for c, ctiles in enumerate(KSPLIT):
            csz = ctiles * P
            for r in range(NCORES):
                ktb = kvst.tile([P, DK, csz], f8, tag=f"kt{c}", name=f"ktb{c}_{r}")
                nc.sync.dma_start(
                    out=ktb,
                    in_=kv_out_k[c][r * D * csz : (r + 1) * D * csz].rearrange(
                        "(k p f) -> p k f", p=P, k=DK
                    ),
                )
                for mj in range(ctiles):
                    kt_i = r * MT + tbase + mj
                    pt = ps.tile([P, TOK], f32, tag="pb")
                    for k2 in range(DK // 2):
                        nc.tensor.matmul(
                            pt,
                            ktb[:, 2 * k2 : 2 * k2 + 2, mj * P : (mj + 1) * P],
                            qT_f8[:, 2 * k2 : 2 * k2 + 2, :],
                            start=(k2 == 0),
                            stop=(k2 == DK // 2 - 1),
                            perf_mode=DR,
                        )
                    nc.scalar.activation(
                        pT_sb[:, kt_i, :], pt, Exp, bias=0.0, scale=SCALE / (WS * WS)
                    )
                    nc.tensor.matmul(
                        psd,
                        ones_f8,
                        pT_sb[:, kt_i, :],
                        start=(c == 0 and r == 0 and mj == 0),
                        stop=(
                            c == len(KSPLIT) - 1
                            and r == NCORES - 1
                            and mj == ctiles - 1
                        ),
                    )
            tbase += ctiles

        # rden = 1/(16*denom), broadcast along partitions via rank-1 MM
        rden = lns.tile([1, TOK], f32, tag="ln")
        nc.vector.reciprocal(rden, psd)
        rden_bp = bc.tile([P, TOK], f32, tag="bc")
        nc.tensor.matmul(rden_bp, ones16_r, rden)
        rden_sb = bcs.tile([P, TOK], f32, tag="bcs")
        nc.scalar.copy(rden_sb, rden_bp)

        # ---- attention output attnT = V.T @ P^T (fp8 DR), + residual --
        # After each V feature-chunk completes its residual tiles, the
        # corresponding k-slices of the collapsed FFN run immediately
        # (accumulated in SBUF), so only the last chunk's slice remains
        # after the final AllGather lands.
        resb = [
            fts.tile([P, TOK], bf16, name=f"resb{m}", tag=f"resb{m}")
            for m in range(DK)
        ]
        accf = xq.tile([P, DK, TOK], bf16)
        sqb0 = xq.tile([P, DK, TOK], bf16)
        psm0 = pss.tile([1, TOK], f32, tag="psm")
        psq0 = pss.tile([1, TOK], f32, tag="psm")
        m0 = 0
        for c, cm in enumerate(VSPLIT):
            for mi in range(cm):
                m = m0 + mi
                pt = ps.tile([P, TOK], f32, tag="pb")
                for r in range(NCORES):
                    vt = vtst.tile([P, MT, P], f8, tag="vt")
                    nc.sync.dma_start(
                        out=vt,
                        in_=kv_out_v[c][
                            (r * cm + mi) * MBLK : (r * cm + mi + 1) * MBLK
                        ].rearrange("(p t f) -> p t f", p=P, t=MT),
                    )
                    for tp in range(MT // 2):
                        kt_i = r * MT + 2 * tp
                        nc.tensor.matmul(
                            pt,
                            vt[:, 2 * tp : 2 * tp + 2, :],
                            pT_sb[:, kt_i : kt_i + 2, :],
                            start=(r == 0 and tp == 0),
                            stop=(r == NCORES - 1 and tp == MT // 2 - 1),
                            perf_mode=DR,
                        )
                # psum-reading mul on DVE (gpsimd has no PSUM port); the
                # residual add + square run on gpsimd in parallel
                tmp = ev.tile([P, TOK], f32, tag="ev")
                nc.vector.tensor_mul(tmp, pt, rden_sb)
                nc.gpsimd.tensor_add(resb[m][:], tmp, xT_sb[:, m, :])
                nc.gpsimd.tensor_mul(sqb0[:, m, :], resb[m][:], resb[m][:])
            # FFN k-slices for this chunk's freshly finished resb tiles
            for m in range(DK):
                pt2 = ps.tile([P, TOK], f32, tag="pb")
                for ki in range(cm):
                    k = m0 + ki
                    nc.tensor.matmul(
                        pt2,
                        wff_sb[:, m * DK + k, :],
                        resb[k][:],
                        start=(ki == 0),
                        stop=(ki == cm - 1),
                    )
                if c == 0:
                    nc.vector.tensor_copy(accf[:, m, :], pt2)
                else:
                    nc.vector.tensor_add(accf[:, m, :], accf[:, m, :], pt2)
            # this chunk's LN0 stats, batched (no MM<->DVE ping-pong)
            for mi in range(cm):
                m = m0 + mi
                nc.tensor.matmul(
                    psm0, onesd_b, resb[m][:], start=(m == 0), stop=(m == DK - 1)
                )
                nc.tensor.matmul(
                    psq0,
                    onesd_b,
                    sqb0[:, m, :],
                    start=(m == 0),
                    stop=(m == DK - 1),
                )
            m0 += cm

        # ---- LN0 stats finalize: rstd0 / mu0*rstd0 broadcasts ---------
        def ln_chain(psm, psq):
            mu2 = lns.tile([1, TOK], f32, tag="ln")
            nc.scalar.square(mu2, psm)
            var = lns.tile([1, TOK], f32, tag="ln")
            nc.vector.tensor_sub(var, psq, mu2)
            std = lns.tile([1, TOK], f32, tag="ln")
            nc.scalar.activation(std, var, Sqrt, bias=eps_sb[:])
            rstd = lns.tile([1, TOK], f32, tag="ln")
            nc.vector.reciprocal(rstd, std)
            msr = lns.tile([1, TOK], f32, tag="ln")
            nc.vector.tensor_mul(msr, psm, rstd)
            rstd_bp = bc.tile([P, TOK], f32, tag="bc")
            nc.tensor.matmul(rstd_bp, ones1_r, rstd)
            msr_bp = bc.tile([P, TOK], f32, tag="bc")
            nc.tensor.matmul(msr_bp, ones1_r, msr)
            return rstd_bp, msr_bp

        rstd0_bp, msr0_bp = ln_chain(psm0, psq0)
        rstd0_sb = bcs.tile([P, TOK], f32, tag="bcs")
        nc.scalar.copy(rstd0_sb, rstd0_bp)

        # ---- FFN finalize (accf already holds Wff@res) + LN1 stats ----
        # out1 = accf*rstd0 + (msr0*wcol + cb), fused into two ops
        out1b = xq.tile([P, DK, TOK], bf16)
        sqb1 = sqb0  # LN0 stats consumed by now; reuse the buffer
        psm1 = pss.tile([1, TOK], f32, tag="psm")
        psq1 = pss.tile([1, TOK], f32, tag="psm")
        for m in range(DK):
            t1 = ev.tile([P, TOK], f32, tag="ev")
            nc.gpsimd.tensor_mul(t1, accf[:, m, :], rstd0_sb)
            nc.vector.affine_then_add(
                out1b[:, m, :],
                msr0_bp,
                t1,
                wcol_sb[:, m : m + 1],
                cb_sb[:, m : m + 1],
            )
            nc.vector.tensor_mul(sqb1[:, m, :], out1b[:, m, :], out1b[:, m, :])
        for m in range(DK):
            nc.tensor.matmul(
                psm1, onesd_b, out1b[:, m, :], start=(m == 0), stop=(m == DK - 1)
            )
            nc.tensor.matmul(
                psq1, onesd_b, sqb1[:, m, :], start=(m == 0), stop=(m == DK - 1)
            )

        # ---- final layernorm + writeback ------------------------------
        # out = (out1*g1)*rstd1 + (msr1*(-g1) + b1n), two fused DVE ops
        rstd1_bp, msr1_bp = ln_chain(psm1, psq1)
        ng1_sb = consts.tile([P, DK], f32)
        nc.vector.tensor_scalar_mul(ng1_sb, g1_sb, -1.0)
        out_re = outT.rearrange("(m p) f -> p m f", p=P)
        for m in range(DK):
            t1 = ev.tile([P, TOK], f32, tag="ev")
            a1 = ev.tile([P, 1], f32, tag="a1")
            nc.vector.affine_mul_reduce(
                t1, a1, out1b[:, m, :], rstd1_bp, g1_sb[:, m : m + 1], 0.0
            )
            ot = ev.tile([P, TOK], f32, tag="ev")
            nc.vector.affine_then_add(
                ot,
                msr1_bp,
                t1,
                ng1_sb[:, m : m + 1],
                b1n_sb[:, m : m + 1],
            )
            dmaq[m % 2].dma_start(out=out_re[:, m, :], in_=ot)

    nc.finalize()
    return nc


def _get_nc():
    if "nc" not in _cache:
        _cache["nc"] = _build_nc()
    return _cache["nc"]


def _make_in_maps(inputs):
    import ml_dtypes

    bf = ml_dtypes.bfloat16
    f8 = ml_dtypes.float8_e4m3
    x = np.ascontiguousarray(np.asarray(inputs["x"], dtype=np.float32))
    Wq = np.asarray(inputs["Wq"], np.float64)
    Wk = np.asarray(inputs["Wk"], np.float64)
    Wv = np.asarray(inputs["Wv"], np.float64)
    W1 = np.asarray(inputs["W1"], np.float64)
    W2 = np.asarray(inputs["W2"], np.float64)
    g0 = np.asarray(inputs["g0"], np.float64)
    b0 = np.asarray(inputs["b0"], np.float64)
    b1 = np.asarray(inputs["b1"], np.float64)
    b2 = np.asarray(inputs["b2"], np.float64)
    bv = np.asarray(inputs["bv"], np.float64)
    # FFN has no activation between the Linears: collapse + LN0 fold.
    Mi = W2 @ W1 + np.eye(D)
    Wff = Mi * g0[None, :]

    def to_f8(a):
        return np.ascontiguousarray(
            np.clip(a, -240.0, 240.0).astype(np.float32)
        ).astype(f8)

    shared = {
        "WqT": to_f8((WS * Wq).T),
        "WkT": to_f8((WS * Wk).T),
        "WvT": to_f8((WS * Wv).T),
        "WffT": np.ascontiguousarray(Wff.T.astype(np.float32)).astype(bf),
        "bv16": np.ascontiguousarray((WS * bv).astype(np.float32)),
        "wcol": np.ascontiguousarray((-(Mi @ g0)).astype(np.float32)),
        "cbv": np.ascontiguousarray((Mi @ b0 + W2 @ b1 + b2).astype(np.float32)),
        "g1": np.ascontiguousarray(np.asarray(inputs["g1"], np.float32)),
        "b1n": np.ascontiguousarray(np.asarray(inputs["b1n"], np.float32)),
    }
    in_maps = []
    for c in range(NCORES):
        m = dict(shared)
        m["xT"] = np.ascontiguousarray(x[c * TOK : (c + 1) * TOK, :].T)
        in_maps.append(m)
    return in_maps


def _assemble(res):
    out = np.empty((N, D), dtype=np.float32)
    for c in range(NCORES):
        out[c * TOK : (c + 1) * TOK, :] = res.results[c]["outT"].T
    return out


def kernel(**inputs):
    from concourse import bass_utils

    nc = _get_nc()
    res = bass_utils.run_bass_kernel_spmd(
        nc, _make_in_maps(inputs), core_ids=list(range(NCORES)), trace=False
    )
    return _assemble(res)


def run_traced(inputs):
    """Like kernel() but with NTFF tracing; returns (out, exec_time_ns, results)."""
    import hookshim

    hookshim.install()
    from concourse import bass_utils

    nc = _get_nc()
    res = bass_utils.run_bass_kernel_spmd(
        nc, _make_in_maps(inputs), core_ids=list(range(NCORES)), trace=True
    )
    return _assemble(res), res.exec_time_ns, res
